# revision 16
# baseline (speedup 1.0000x reference)
"""Bahdanau-attention kernel for Trainium2 (8 NeuronCores, Bass/Tile).

Computation (reference, fp32):
    Wh  = hidden @ W_w.T + W_b                      # [B, H]
    Ue  = einsum('bse,he->bsh', enc^T, U_w) + U_b   # [B, S, H]
    en  = tanh(Wh[:,None,:] + Ue) @ v_w[0]          # [B, S]
    out = softmax(where(mask, -1e10, en), axis=1)

Strategy
- Data-parallel over batch: 8 rows per core, weights replicated.
- Masked positions contribute exactly 0 to the softmax (exp(-1e10)=0 in
  fp32), so the host packs only the unmasked s-columns per row and
  scatters results back (exact, not an approximation).  Rows are sorted
  by unmasked count and rank-grouped into 8 slots (one row per core per
  slot) so each slot's padded width is the max of 8 similar counts.
- The packed columns form one flat [0,TOT) space per core, processed in
  balanced mult-of-16 blocks (<=496 wide, smallest last) that ignore
  slot boundaries: the main GEMM, the v-weighting, the partition-reduce
  and the exp are all slot-blind; only the tanh bias is per slot
  segment.
- Main GEMM in fp8 (e4m3) with DoubleRow perf mode: 2 e-rows per PE
  cell, 256-deep contraction per matmul, 8 matmuls per (block,
  h-chunk).  U_w is scaled by 256 before quantization (its entries are
  subnormal in e4m3 otherwise) and the tanh ACT un-scales by 1/256.
  Measured: N-wide DR matmuls stream at N/2.4GHz + ~2.5ns with the
  LDWEIGHTS fully hidden on the parallel weight-load path, so the
  block-outer loop is already at the streaming roofline.
- Wh + W_b + U_b is precomputed on the host (f32) and shipped as a
  [128, HC*BL] bias table applied inside the tanh ACT per slot segment.
  This removes the on-device Wh GEMM, its W_w DMA stream and the filler
  matmuls the previous version needed to pace it.
- A short dummy-matmul burst on scratch SBUF warms the PE HAM clock
  gate during the DMA ramp; real matmuls start as soon as the first
  enc quarter + uw chunk land.
- v-weighting runs fused on the Vector engine (scalar_tensor_tensor:
  acc = th*v + acc, one op per (block,hc)); a f32 ones-vector M=1
  matmul per block does the partition reduce (off critical path).  The
  LAST (smallest) block instead reduces via direct lhsT=v-chunk
  matmuls, keeping the tail to tanh+tiny-MM+exp+DMA.
- The device applies exp; softmax normalization (sum+divide per row)
  happens in the host unpack loop.
"""

import numpy as np
import ml_dtypes

B, S, H, E = 64, 512, 1024, 2048
NCORES = 8
BL = B // NCORES          # rows (slots) per core
HC = H // 128             # h chunks
JP = E // 256             # DoubleRow e-chunk pairs
USCALE = 256.0            # fp8 pre-scale for U_w

bf16 = ml_dtypes.bfloat16
f8 = ml_dtypes.float8_e4m3

_CACHE = {}

N_WARM = 12               # dummy warm-up matmuls (N=256) during DMA ramp


def _blocks(TOT):
    """512-wide blocks (the only width where LDWEIGHTS hides fully
    behind the matmul stream); the narrow remainder lands last, giving
    the shortest possible tail chain."""
    assert TOT % 16 == 0
    bs = []
    c = 0
    while c < TOT:
        w = min(512, TOT - c)
        bs.append((c, c + w))
        c += w
    assert all((c1 - c0) % 16 == 0 and c1 - c0 <= 512 for c0, c1 in bs)
    return bs


def _build_nc(Ws):
    """Per-core program; Ws = tuple of 8 slot widths, in processing
    order (descending), sum mult of 16."""
    import concourse.mybir as mybir
    import concourse.tile as tile
    from concourse import bacc

    F32 = mybir.dt.float32
    BF = mybir.dt.bfloat16
    FP8 = mybir.dt.float8e4
    AF = mybir.ActivationFunctionType
    DR = mybir.MatmulPerfMode.DoubleRow
    MUL = mybir.AluOpType.mult
    ADD = mybir.AluOpType.add

    Ws = list(Ws)
    TOT = sum(Ws)
    off = [0]
    for w in Ws:
        off.append(off[-1] + w)
    blocks = _blocks(TOT)
    NB = len(blocks)
    # flat offset of block k in the [p, blk, jp, i, c] enc layout
    fb = [2 * JP * c0 for (c0, c1) in blocks]

    # per-block slot segments: (colr0, colr1, slot) relative to block
    segs = []
    for k, (c0, c1) in enumerate(blocks):
        ss = []
        for b in range(BL):
            lo = max(c0, off[b])
            hi = min(c1, off[b + 1])
            if lo < hi:
                ss.append((lo - c0, hi - c0, b))
        segs.append(ss)

    nc = bacc.Bacc(num_swdge_queues=4)
    # enc packed fp8: [p, blk, jp, i(2), c]  flattened on dim 1
    enc_t = nc.declare_dram_parameter("enc8", [128, 2 * JP * TOT], FP8,
                                      isOutput=False)
    # U_w.T * 256 in fp8: [p(=e%128), hc, jp, i(2), v(=h%128)]
    uwT = nc.declare_dram_parameter("uw8", [128, HC * JP * 2 * 128], FP8,
                                    isOutput=False)
    # v chunks bf16: [p(=h%128), hc]
    vt_d = nc.declare_dram_parameter("vt", [128, HC], BF, isOutput=False)
    # host-side Wh + W_b + U_b [.,0:HC*BL] and f32 v chunks [.,HC*BL:+HC]
    VC0 = HC * BL
    bias_d = nc.declare_dram_parameter("bias", [128, HC * BL + HC], F32,
                                       isOutput=False)
    out_d = nc.declare_dram_parameter("out", [1, TOT], F32, isOutput=True)

    with tile.TileContext(nc) as tc:
        with (
            tc.tile_pool(name="const", bufs=1) as cst,
            tc.tile_pool(name="wpool", bufs=1) as wp,
            tc.tile_pool(name="thp", bufs=6) as thp,
            tc.tile_pool(name="accp", bufs=2) as accp,
            tc.tile_pool(name="pup", bufs=4, space="PSUM") as pup,
            tc.tile_pool(name="pep", bufs=2, space="PSUM") as pep,
            tc.tile_pool(name="pwarm", bufs=1, space="PSUM") as pwarm,
        ):
            # ---- PE warm-up on scratch data (HAM clock-gate release) ---
            warm_sb = cst.tile([128, 384], BF, tag="warm")
            nc.gpsimd.memset(warm_sb[:], 1.0)
            warm_ps = pwarm.tile([128, 256], F32, tag="wps")
            for _ in range(N_WARM):
                nc.tensor.matmul(
                    warm_ps[:], lhsT=warm_sb[:, 0:128], rhs=warm_sb[:, 128:384],
                    start=True, stop=True,
                )

            # ---- tiles ---------------------------------------------------
            vt_sb = cst.tile([128, HC], BF, tag="vt")
            bias_sb = cst.tile([128, HC * BL + HC], F32, tag="bias")
            ww_cols = HC * JP * 2 * 128
            uw_sb = wp.tile([128, ww_cols], FP8, tag="uw8")
            enc_sb = wp.tile([128, 2 * JP * TOT], FP8, tag="enc8")
            res_sb = cst.tile([1, TOT], F32, tag="res")
            ones_sb = cst.tile([128, 1], BF, tag="ones")
            nc.gpsimd.memset(ones_sb[:], 1.0)

            UWC = JP * 2 * 128  # uw8 cols per hc

            def uw_dma(eng, hc):
                a = hc * UWC
                eng.dma_start(uw_sb[:, a:a + UWC], uwT[:, a:a + UWC])

            def enc_dma(eng, k, part, nparts):
                c0, c1 = blocks[k]
                w = 2 * JP * (c1 - c0)
                a = fb[k] + part * w // nparts
                bnd = fb[k] + (part + 1) * w // nparts
                eng.dma_start(enc_sb[:, a:bnd], enc_t[:, a:bnd])

            s, g, sc = nc.sync, nc.gpsimd, nc.scalar
            # Three issue queues in parallel, ordered by first PE need.
            # A [128 x 2KB-row] transfer takes ~3us wall (descriptors
            # shared over the HW queues), so the pieces gating the first
            # matmul are kept small: uw0 split j0/j1/rest, block-0 enc by
            # j-strips.  Remainder + later blocks have large slack.
            s.dma_start(uw_sb[:, 0:256], uwT[:, 0:256])          # uw0 j0
            enc_dma(g, 0, 0, 8)     # j0
            enc_dma(sc, 0, 1, 8)    # j1
            s.dma_start(uw_sb[:, 256:512], uwT[:, 256:512])      # uw0 j1
            enc_dma(g, 0, 2, 8)     # j2
            enc_dma(sc, 0, 3, 8)    # j3
            s.dma_start(uw_sb[:, 512:UWC], uwT[:, 512:UWC])      # uw0 rest
            enc_dma(g, 0, 4, 8)
            enc_dma(sc, 0, 5, 8)
            s.dma_start(vt_sb[:], vt_d[:])
            s.dma_start(bias_sb[:], bias_d[:])
            enc_dma(g, 0, 6, 8)
            enc_dma(sc, 0, 7, 8)
            # uw chunks round-robin across queues by deadline (uw_k is
            # needed ~1.7us*k after block 0 starts); bulk strictly after.
            uw_dma(s, 1)
            uw_dma(g, 2)
            uw_dma(sc, 3)
            uw_dma(s, 4)
            uw_dma(g, 5)
            uw_dma(sc, 6)
            uw_dma(s, 7)
            for k in range(1, NB - 1):
                enc_dma(g if k % 2 else s, k, 0, 2)
                enc_dma(s if k % 2 else g, k, 1, 2)
            enc_dma(g, NB - 1, 0, 1)  # remainder block enc (tiny, slack)

            # ---- main loop over column blocks ---------------------------
            for kidx, (c0, c1) in enumerate(blocks):
                k = kidx
                bw = c1 - c0
                lastb = kidx == NB - 1
                acc = accp.tile([128, 512], F32, tag="acc")
                for hc in range(HC):
                    pu = pup.tile([128, 512], F32, tag="pu")
                    for j in range(JP):
                        rr = enc_sb[:, fb[k] + j * 2 * bw:
                                    fb[k] + (j + 1) * 2 * bw]
                        ll = uw_sb[:, hc * UWC + j * 256:
                                   hc * UWC + j * 256 + 256]
                        nc.tensor.matmul(
                            pu[:, 0:bw],
                            lhsT=ll.rearrange("p (i v) -> p i v", i=2),
                            rhs=rr.rearrange("p (i s) -> p i s", i=2),
                            start=(j == 0),
                            stop=(j == JP - 1),
                            perf_mode=DR,
                        )
                    th = thp.tile([128, 512], BF, tag="th")
                    for (r0, r1, b) in segs[k]:
                        nc.scalar.activation(
                            th[:, r0:r1], pu[:, r0:r1], AF.Tanh,
                            bias=bias_sb[:, hc * BL + b:hc * BL + b + 1],
                            scale=1.0 / USCALE,
                        )
                    # fused acc = th*v + acc on the Vector engine
                    vcol = bias_sb[:, VC0 + hc:VC0 + hc + 1]
                    if hc == 0:
                        nc.vector.tensor_scalar_mul(
                            acc[:, 0:bw], th[:, 0:bw],
                            bias_sb[:, VC0:VC0 + 1])
                    else:
                        nc.vector.scalar_tensor_tensor(
                            acc[:, 0:bw], th[:, 0:bw], vcol,
                            acc[:, 0:bw], MUL, ADD)
                # bf16 partition-reduce matmul (an f32 one lowers to two
                # ~615ns instructions; cast+bf16 is ~3x cheaper PE-side)
                accb = thp.tile([128, 512], BF, tag="accb")
                nc.vector.tensor_copy(accb[:, 0:bw], acc[:, 0:bw])
                pe_ = pep.tile([1, 512], F32, tag="pe")
                nc.tensor.matmul(
                    pe_[0:1, 0:bw], lhsT=ones_sb[:, 0:1],
                    rhs=accb[:, 0:bw], start=True, stop=True,
                )

                # ---- exp over the block; normalization on host ----------
                nc.scalar.activation(res_sb[0:1, c0:c1], pe_[0:1, 0:bw],
                                     AF.Exp)
                nc.sync.dma_start(out_d[0:1, c0:c1], res_sb[0:1, c0:c1])

    nc.finalize()
    return nc


def _prep_inputs(hidden, encoder_outputs, mask, W_w, W_b, U_w, U_b, v_w):
    enc_bf = encoder_outputs.astype(bf16)          # [S, B, E]
    # U_w.T * 256 -> fp8 : [p(=e%128), hc, jp, i, v(=h%128)]
    uwT_np = (np.ascontiguousarray(U_w.T) * USCALE).astype(f8)   # [E, H]
    uwT_np = np.ascontiguousarray(
        uwT_np.reshape(JP, 2, 128, HC, 128).transpose(2, 3, 0, 1, 4)
    ).reshape(128, HC * JP * 2 * 128)
    vt_np = np.ascontiguousarray(v_w[0].reshape(HC, 128).T).astype(bf16)

    # host-side Wh + W_b + U_b (exact f32)
    WhU = (hidden.astype(np.float64) @ W_w.T.astype(np.float64)
           + W_b.astype(np.float64) + U_b.astype(np.float64)
           ).astype(np.float32)                                  # [B, H]

    idx_all = [np.nonzero(~mask[i])[0] for i in range(B)]
    counts = np.array([len(ix) for ix in idx_all])

    # sorted-slot packing: rank-group rows into 8 slots of 8 (one per core)
    order = np.argsort(-counts, kind="stable")
    rows = order.reshape(BL, NCORES)       # rows[b][c] = global row index
    Ws = [int(max(4, counts[rows[b]].max())) for b in range(BL)]
    Ws[-1] += (-sum(Ws)) % 16              # pad TOT to a mult of 16
    Ws = tuple(Ws)
    TOT = sum(Ws)
    off = np.concatenate([[0], np.cumsum(Ws)]).astype(int)
    blocks = _blocks(TOT)

    in_maps = []
    for c in range(NCORES):
        crows = rows[:, c]                                       # slot -> row
        # enc for this core's rows: [E, BL, S]
        enc_c = np.ascontiguousarray(enc_bf[:, crows, :].transpose(2, 1, 0))
        enc_flat = np.zeros((E, TOT), np.float32)
        for b in range(BL):
            ix = idx_all[crows[b]]
            cnt = len(ix)
            if cnt:
                enc_flat[:, off[b]:off[b] + cnt] = enc_c[:, b, ix]
        enc8 = enc_flat.astype(f8)                               # [E, TOT]
        # [E, TOT] -> [jp, i, p, col] -> [p, jp, i, col]
        enc8 = enc8.reshape(JP, 2, 128, TOT).transpose(2, 0, 1, 3)
        parts = [np.ascontiguousarray(enc8[:, :, :, c0:c1]).reshape(128, -1)
                 for (c0, c1) in blocks]
        enc_p = np.ascontiguousarray(np.concatenate(parts, axis=1))
        # bias[p, hc*BL + b] = WhU[row(b), hc*128 + p]; then f32 v chunks
        bias_c = np.ascontiguousarray(
            WhU[crows].reshape(BL, HC, 128).transpose(2, 1, 0)
        ).reshape(128, HC * BL)
        bias_c = np.concatenate(
            [bias_c, v_w[0].reshape(HC, 128).T.astype(np.float32)], axis=1)
        bias_c = np.ascontiguousarray(bias_c)
        in_maps.append({
            "enc8": enc_p,
            "uw8": uwT_np,
            "vt": vt_np,
            "bias": bias_c,
        })
    return in_maps, Ws, rows, idx_all, counts


def _run(in_maps, Ws, trace=False):
    from concourse import bass_utils
    if Ws not in _CACHE:
        _CACHE[Ws] = _build_nc(Ws)
    nc = _CACHE[Ws]
    return bass_utils.run_bass_kernel_spmd(
        nc, in_maps, core_ids=list(range(NCORES)), trace=trace
    )


def kernel(hidden, encoder_outputs, mask, W_w, W_b, U_w, U_b, v_w,
           _trace=False, _return_bkr=False):
    hidden = np.asarray(hidden, dtype=np.float32)
    encoder_outputs = np.asarray(encoder_outputs, dtype=np.float32)
    mask = np.asarray(mask).astype(bool)
    W_w = np.asarray(W_w, dtype=np.float32)
    W_b = np.asarray(W_b, dtype=np.float32)
    U_w = np.asarray(U_w, dtype=np.float32)
    U_b = np.asarray(U_b, dtype=np.float32)
    v_w = np.asarray(v_w, dtype=np.float32)

    in_maps, Ws, rows, idx_all, counts = _prep_inputs(
        hidden, encoder_outputs, mask, W_w, W_b, U_w, U_b, v_w)
    bkr = _run(in_maps, Ws, trace=_trace)

    offs = np.concatenate([[0], np.cumsum(Ws)]).astype(int)
    out = np.zeros((B, S), np.float32)
    for c in range(NCORES):
        dev = bkr.results[c]["out"].reshape(-1)
        for b in range(BL):
            i = rows[b, c]
            cnt = counts[i]
            if cnt:
                e = dev[offs[b]:offs[b] + cnt]
                out[i, idx_all[i]] = e / e.sum()
            else:
                # fully-masked row: softmax over all -1e10 is uniform
                out[i, :] = np.float32(1.0 / S)
    if _return_bkr:
        return out, bkr
    return out


# revision 17
# speedup vs baseline: 1.1551x; 1.1551x over previous
"""Bahdanau-attention kernel for Trainium2 (8 NeuronCores, Bass/Tile).

Computation (reference, fp32):
    Wh  = hidden @ W_w.T + W_b                      # [B, H]
    Ue  = einsum('bse,he->bsh', enc^T, U_w) + U_b   # [B, S, H]
    en  = tanh(Wh[:,None,:] + Ue) @ v_w[0]          # [B, S]
    out = softmax(where(mask, -1e10, en), axis=1)

Strategy
- Data-parallel over batch: 8 rows per core, weights replicated.
- Masked positions contribute exactly 0 to the softmax (exp(-1e10)=0 in
  fp32), so the host packs only the unmasked s-columns per row and
  scatters results back (exact, not an approximation).  Rows are sorted
  by unmasked count and rank-grouped into 8 slots (one row per core per
  slot) so each slot's padded width is the max of 8 similar counts.
- The packed columns form one flat [0,TOT) space per core, processed in
  balanced mult-of-16 blocks (<=496 wide, smallest last) that ignore
  slot boundaries: the main GEMM, the v-weighting, the partition-reduce
  and the exp are all slot-blind; only the tanh bias is per slot
  segment.
- Main GEMM in fp8 (e4m3) with DoubleRow perf mode: 2 e-rows per PE
  cell, 256-deep contraction per matmul, 8 matmuls per (block,
  h-chunk).  U_w is scaled by 256 before quantization (its entries are
  subnormal in e4m3 otherwise) and the tanh ACT un-scales by 1/256.
  Measured: N-wide DR matmuls stream at N/2.4GHz + ~2.5ns with the
  LDWEIGHTS fully hidden on the parallel weight-load path, so the
  block-outer loop is already at the streaming roofline.
- Wh + W_b + U_b is precomputed on the host (f32) and shipped as a
  [128, HC*BL] bias table applied inside the tanh ACT per slot segment.
  This removes the on-device Wh GEMM, its W_w DMA stream and the filler
  matmuls the previous version needed to pace it.
- A short dummy-matmul burst on scratch SBUF warms the PE HAM clock
  gate during the DMA ramp; real matmuls start as soon as the first
  enc quarter + uw chunk land.
- v-weighting runs fused on the Vector engine (scalar_tensor_tensor:
  acc = th*v + acc, one op per (block,hc)); a f32 ones-vector M=1
  matmul per block does the partition reduce (off critical path).  The
  LAST (smallest) block instead reduces via direct lhsT=v-chunk
  matmuls, keeping the tail to tanh+tiny-MM+exp+DMA.
- The device applies exp; softmax normalization (sum+divide per row)
  happens in the host unpack loop.
"""

import numpy as np
import ml_dtypes

B, S, H, E = 64, 512, 1024, 2048
NCORES = 8
BL = B // NCORES          # rows (slots) per core
HC = H // 128             # h chunks
JP = E // 256             # DoubleRow e-chunk pairs
USCALE = 256.0            # fp8 pre-scale for U_w

bf16 = ml_dtypes.bfloat16
f8 = ml_dtypes.float8_e4m3

_CACHE = {}

N_WARM = 12               # dummy warm-up matmuls (N=256) during DMA ramp


def _blocks(TOT):
    """512-wide blocks (the only width where LDWEIGHTS hides fully
    behind the matmul stream); the narrow remainder lands last, giving
    the shortest possible tail chain."""
    assert TOT % 16 == 0
    bs = []
    c = 0
    while c < TOT:
        w = min(512, TOT - c)
        bs.append((c, c + w))
        c += w
    assert all((c1 - c0) % 16 == 0 and c1 - c0 <= 512 for c0, c1 in bs)
    return bs


def _build_nc(Ws):
    """Per-core program; Ws = tuple of 8 slot widths, in processing
    order (descending), sum mult of 16."""
    import concourse.mybir as mybir
    import concourse.tile as tile
    from concourse import bacc

    F32 = mybir.dt.float32
    BF = mybir.dt.bfloat16
    FP8 = mybir.dt.float8e4
    AF = mybir.ActivationFunctionType
    DR = mybir.MatmulPerfMode.DoubleRow
    MUL = mybir.AluOpType.mult
    ADD = mybir.AluOpType.add

    Ws = list(Ws)
    TOT = sum(Ws)
    off = [0]
    for w in Ws:
        off.append(off[-1] + w)
    blocks = _blocks(TOT)
    NB = len(blocks)
    # flat offset of block k in the [p, blk, jp, i, c] enc layout
    fb = [2 * JP * c0 for (c0, c1) in blocks]

    # per-block slot segments: (colr0, colr1, slot) relative to block
    segs = []
    for k, (c0, c1) in enumerate(blocks):
        ss = []
        for b in range(BL):
            lo = max(c0, off[b])
            hi = min(c1, off[b + 1])
            if lo < hi:
                ss.append((lo - c0, hi - c0, b))
        segs.append(ss)

    nc = bacc.Bacc(num_swdge_queues=4)
    # enc packed fp8: [p, blk, jp, i(2), c]  flattened on dim 1
    enc_t = nc.declare_dram_parameter("enc8", [128, 2 * JP * TOT], FP8,
                                      isOutput=False)
    # U_w.T * 256 in fp8: [p(=e%128), hc, jp, i(2), v(=h%128)]
    uwT = nc.declare_dram_parameter("uw8", [128, HC * JP * 2 * 128], FP8,
                                    isOutput=False)
    # v chunks bf16: [p(=h%128), hc]
    vt_d = nc.declare_dram_parameter("vt", [128, HC], BF, isOutput=False)
    # host-side Wh + W_b + U_b [.,0:HC*BL] and f32 v chunks [.,HC*BL:+HC]
    VC0 = HC * BL
    bias_d = nc.declare_dram_parameter("bias", [128, HC * BL + HC], F32,
                                       isOutput=False)
    out_d = nc.declare_dram_parameter("out", [1, TOT], F32, isOutput=True)

    with tile.TileContext(nc) as tc:
        with (
            tc.tile_pool(name="const", bufs=1) as cst,
            tc.tile_pool(name="wpool", bufs=1) as wp,
            tc.tile_pool(name="thp", bufs=6) as thp,
            tc.tile_pool(name="accp", bufs=2) as accp,
            tc.tile_pool(name="pup", bufs=4, space="PSUM") as pup,
            tc.tile_pool(name="pep", bufs=2, space="PSUM") as pep,
            tc.tile_pool(name="pwarm", bufs=1, space="PSUM") as pwarm,
        ):
            # ---- PE warm-up on scratch data (HAM clock-gate release) ---
            warm_sb = cst.tile([128, 384], BF, tag="warm")
            nc.gpsimd.memset(warm_sb[:], 1.0)
            warm_ps = pwarm.tile([128, 256], F32, tag="wps")
            for _ in range(N_WARM):
                nc.tensor.matmul(
                    warm_ps[:], lhsT=warm_sb[:, 0:128], rhs=warm_sb[:, 128:384],
                    start=True, stop=True,
                )

            # ---- tiles ---------------------------------------------------
            vt_sb = cst.tile([128, HC], BF, tag="vt")
            bias_sb = cst.tile([128, HC * BL + HC], F32, tag="bias")
            ww_cols = HC * JP * 2 * 128
            uw_sb = wp.tile([128, ww_cols], FP8, tag="uw8")
            enc_sb = wp.tile([128, 2 * JP * TOT], FP8, tag="enc8")
            res_sb = cst.tile([1, TOT], F32, tag="res")
            ones_sb = cst.tile([128, 1], BF, tag="ones")
            nc.gpsimd.memset(ones_sb[:], 1.0)

            UWC = JP * 2 * 128  # uw8 cols per hc

            def uw_dma(eng, hc):
                a = hc * UWC
                eng.dma_start(uw_sb[:, a:a + UWC], uwT[:, a:a + UWC])

            def enc_dma(eng, k, part, nparts):
                c0, c1 = blocks[k]
                w = 2 * JP * (c1 - c0)
                a = fb[k] + part * w // nparts
                bnd = fb[k] + (part + 1) * w // nparts
                eng.dma_start(enc_sb[:, a:bnd], enc_t[:, a:bnd])

            s, g, sc = nc.sync, nc.gpsimd, nc.scalar
            # Three issue queues in parallel, ordered by first PE need.
            # A [128 x 2KB-row] transfer takes ~3us wall (descriptors
            # shared over the HW queues), so the pieces gating the first
            # matmul are kept small: uw0 split j0/j1/rest, block-0 enc by
            # j-strips.  Remainder + later blocks have large slack.
            s.dma_start(uw_sb[:, 0:256], uwT[:, 0:256])          # uw0 j0
            enc_dma(g, 0, 0, 8)     # j0
            enc_dma(sc, 0, 1, 8)    # j1
            s.dma_start(uw_sb[:, 256:512], uwT[:, 256:512])      # uw0 j1
            enc_dma(g, 0, 2, 8)     # j2
            enc_dma(sc, 0, 3, 8)    # j3
            s.dma_start(uw_sb[:, 512:UWC], uwT[:, 512:UWC])      # uw0 rest
            enc_dma(g, 0, 4, 8)
            enc_dma(sc, 0, 5, 8)
            s.dma_start(vt_sb[:], vt_d[:])
            s.dma_start(bias_sb[:], bias_d[:])
            enc_dma(g, 0, 6, 8)
            enc_dma(sc, 0, 7, 8)
            uw_dma(s, 1)
            uw_dma(sc, 2)
            uw_dma(g, 3)
            uw_dma(sc, 4)
            uw_dma(s, 5)
            uw_dma(g, 6)
            uw_dma(s, 7)
            enc_dma(g, NB - 1, 0, 1)  # remainder block enc (tiny)
            for k in range(1, NB - 1):
                enc_dma(s, k, 0, 2)
                enc_dma(g, k, 1, 2)

            # ---- main loop over column blocks ---------------------------
            for kidx, (c0, c1) in enumerate(blocks):
                k = kidx
                bw = c1 - c0
                lastb = kidx == NB - 1
                acc = accp.tile([128, 512], F32, tag="acc")
                for hc in range(HC):
                    pu = pup.tile([128, 512], F32, tag="pu")
                    for j in range(JP):
                        rr = enc_sb[:, fb[k] + j * 2 * bw:
                                    fb[k] + (j + 1) * 2 * bw]
                        ll = uw_sb[:, hc * UWC + j * 256:
                                   hc * UWC + j * 256 + 256]
                        nc.tensor.matmul(
                            pu[:, 0:bw],
                            lhsT=ll.rearrange("p (i v) -> p i v", i=2),
                            rhs=rr.rearrange("p (i s) -> p i s", i=2),
                            start=(j == 0),
                            stop=(j == JP - 1),
                            perf_mode=DR,
                        )
                    th = thp.tile([128, 512], BF, tag="th")
                    for (r0, r1, b) in segs[k]:
                        nc.scalar.activation(
                            th[:, r0:r1], pu[:, r0:r1], AF.Tanh,
                            bias=bias_sb[:, hc * BL + b:hc * BL + b + 1],
                            scale=1.0 / USCALE,
                        )
                    # fused acc = th*v + acc on the Vector engine
                    vcol = bias_sb[:, VC0 + hc:VC0 + hc + 1]
                    if hc == 0:
                        nc.vector.tensor_scalar_mul(
                            acc[:, 0:bw], th[:, 0:bw],
                            bias_sb[:, VC0:VC0 + 1])
                    else:
                        nc.vector.scalar_tensor_tensor(
                            acc[:, 0:bw], th[:, 0:bw], vcol,
                            acc[:, 0:bw], MUL, ADD)
                # bf16 partition-reduce matmul (an f32 one lowers to two
                # ~615ns instructions; cast+bf16 is ~3x cheaper PE-side)
                accb = thp.tile([128, 512], BF, tag="accb")
                nc.vector.tensor_copy(accb[:, 0:bw], acc[:, 0:bw])
                pe_ = pep.tile([1, 512], F32, tag="pe")
                nc.tensor.matmul(
                    pe_[0:1, 0:bw], lhsT=ones_sb[:, 0:1],
                    rhs=accb[:, 0:bw], start=True, stop=True,
                )

                # ---- exp over the block; normalization on host ----------
                nc.scalar.activation(res_sb[0:1, c0:c1], pe_[0:1, 0:bw],
                                     AF.Exp)
                nc.sync.dma_start(out_d[0:1, c0:c1], res_sb[0:1, c0:c1])

    nc.finalize()
    return nc


def _prep_inputs(hidden, encoder_outputs, mask, W_w, W_b, U_w, U_b, v_w):
    enc_bf = encoder_outputs.astype(bf16)          # [S, B, E]
    # U_w.T * 256 -> fp8 : [p(=e%128), hc, jp, i, v(=h%128)]
    uwT_np = (np.ascontiguousarray(U_w.T) * USCALE).astype(f8)   # [E, H]
    uwT_np = np.ascontiguousarray(
        uwT_np.reshape(JP, 2, 128, HC, 128).transpose(2, 3, 0, 1, 4)
    ).reshape(128, HC * JP * 2 * 128)
    vt_np = np.ascontiguousarray(v_w[0].reshape(HC, 128).T).astype(bf16)

    # host-side Wh + W_b + U_b (exact f32)
    WhU = (hidden.astype(np.float64) @ W_w.T.astype(np.float64)
           + W_b.astype(np.float64) + U_b.astype(np.float64)
           ).astype(np.float32)                                  # [B, H]

    idx_all = [np.nonzero(~mask[i])[0] for i in range(B)]
    counts = np.array([len(ix) for ix in idx_all])

    # sorted-slot packing: rank-group rows into 8 slots of 8 (one per core)
    order = np.argsort(-counts, kind="stable")
    rows = order.reshape(BL, NCORES)       # rows[b][c] = global row index
    Ws = [int(max(4, counts[rows[b]].max())) for b in range(BL)]
    Ws[-1] += (-sum(Ws)) % 16              # pad TOT to a mult of 16
    Ws = tuple(Ws)
    TOT = sum(Ws)
    off = np.concatenate([[0], np.cumsum(Ws)]).astype(int)
    blocks = _blocks(TOT)

    in_maps = []
    for c in range(NCORES):
        crows = rows[:, c]                                       # slot -> row
        # enc for this core's rows: [E, BL, S]
        enc_c = np.ascontiguousarray(enc_bf[:, crows, :].transpose(2, 1, 0))
        enc_flat = np.zeros((E, TOT), np.float32)
        for b in range(BL):
            ix = idx_all[crows[b]]
            cnt = len(ix)
            if cnt:
                enc_flat[:, off[b]:off[b] + cnt] = enc_c[:, b, ix]
        enc8 = enc_flat.astype(f8)                               # [E, TOT]
        # [E, TOT] -> [jp, i, p, col] -> [p, jp, i, col]
        enc8 = enc8.reshape(JP, 2, 128, TOT).transpose(2, 0, 1, 3)
        parts = [np.ascontiguousarray(enc8[:, :, :, c0:c1]).reshape(128, -1)
                 for (c0, c1) in blocks]
        enc_p = np.ascontiguousarray(np.concatenate(parts, axis=1))
        # bias[p, hc*BL + b] = WhU[row(b), hc*128 + p]; then f32 v chunks
        bias_c = np.ascontiguousarray(
            WhU[crows].reshape(BL, HC, 128).transpose(2, 1, 0)
        ).reshape(128, HC * BL)
        bias_c = np.concatenate(
            [bias_c, v_w[0].reshape(HC, 128).T.astype(np.float32)], axis=1)
        bias_c = np.ascontiguousarray(bias_c)
        in_maps.append({
            "enc8": enc_p,
            "uw8": uwT_np,
            "vt": vt_np,
            "bias": bias_c,
        })
    return in_maps, Ws, rows, idx_all, counts


def _run(in_maps, Ws, trace=False):
    from concourse import bass_utils
    if Ws not in _CACHE:
        _CACHE[Ws] = _build_nc(Ws)
    nc = _CACHE[Ws]
    return bass_utils.run_bass_kernel_spmd(
        nc, in_maps, core_ids=list(range(NCORES)), trace=trace
    )


def kernel(hidden, encoder_outputs, mask, W_w, W_b, U_w, U_b, v_w,
           _trace=False, _return_bkr=False):
    hidden = np.asarray(hidden, dtype=np.float32)
    encoder_outputs = np.asarray(encoder_outputs, dtype=np.float32)
    mask = np.asarray(mask).astype(bool)
    W_w = np.asarray(W_w, dtype=np.float32)
    W_b = np.asarray(W_b, dtype=np.float32)
    U_w = np.asarray(U_w, dtype=np.float32)
    U_b = np.asarray(U_b, dtype=np.float32)
    v_w = np.asarray(v_w, dtype=np.float32)

    in_maps, Ws, rows, idx_all, counts = _prep_inputs(
        hidden, encoder_outputs, mask, W_w, W_b, U_w, U_b, v_w)
    bkr = _run(in_maps, Ws, trace=_trace)

    offs = np.concatenate([[0], np.cumsum(Ws)]).astype(int)
    out = np.zeros((B, S), np.float32)
    for c in range(NCORES):
        dev = bkr.results[c]["out"].reshape(-1)
        for b in range(BL):
            i = rows[b, c]
            cnt = counts[i]
            if cnt:
                e = dev[offs[b]:offs[b] + cnt]
                out[i, idx_all[i]] = e / e.sum()
            else:
                # fully-masked row: softmax over all -1e10 is uniform
                out[i, :] = np.float32(1.0 / S)
    if _return_bkr:
        return out, bkr
    return out
